# revision 20
# baseline (speedup 1.0000x reference)
"""Trainium2 Bass kernel for EpidemicDynamics: y = 0.1 * x * (A @ (1 - x)).

A is [16384, 16384] f32 (1 GiB) -> memory-bound matvec. Sharding: row-shard A
across 8 NeuronCores (contiguous [2048, 16384] slices), replicate x. Each core
computes its 2048 output rows locally; host concatenates. No collectives.

Per-core dataflow (v2):
  - x arrives once as a [1, 16384] row (64 KiB DMA). A PE outer-product
    (ones[1,128].T @ x_chunk[1,512]) broadcasts it to all 128 partitions in
    PSUM, and ACT copies PSUM->SBUF fused with w = 1 - x. This avoids an
    8.4 MB broadcast read from HBM.
  - partition p owns rows p*16 + t (t=0..15), so the per-row x/y vectors are
    contiguous 64 B runs per partition (cheap DMA descriptors).
  - the A slice streams as 64 tiles of [128 rows, 4096 cols] (2 MiB DMAs);
    each tile takes one DVE scalar_tensor_tensor: product (A * R) * w written
    to a free-step-0 dummy, accum_out = per-partition row sum.
  - finale: y = x * acc (R already folded in), via small DVE ops.
"""

import numpy as np

import concourse.bacc as bacc
import concourse.bass as bass
import concourse.mybir as mybir
import concourse.tile as tile
from concourse.bass_utils import run_bass_kernel_spmd

N = 16384          # problem size (hardcoded per harness contract)
NCORES = 8
ROWS = N // NCORES  # 2048 rows per core
P = 128             # SBUF partitions
NT = ROWS // P      # 16 rows per partition
CHUNK = 4096        # columns per A tile
NCH = N // CHUNK    # 4 chunks per row group
BC = 512            # one matmul's N (one PSUM bank)
PSB = 2048          # PSUM staging tile columns (4 banks); one ACT copy each
XP = 4096           # x row piece held in SBUF
R_COEF = 0.1

F32 = mybir.dt.float32


def build():
    nc = bacc.Bacc()
    A_s = nc.declare_dram_parameter("A_s", [ROWS, N], F32, isOutput=False)
    x_full = nc.declare_dram_parameter("x_full", [N, 1], F32, isOutput=False)
    x_s = nc.declare_dram_parameter("x_s", [ROWS, 1], F32, isOutput=False)
    y_s = nc.declare_dram_parameter("y_s", [ROWS, 1], F32, isOutput=True)

    # partition p <-> rows p*NT + t: [128, CHUNK] tiles with row stride NT*N
    A_r = A_s.rearrange("(p t) n -> t p n", t=NT)
    x_row = x_full.rearrange("n o -> o n")  # [1, N]

    with tile.TileContext(nc) as tc:
        with (
            tc.tile_pool(name="singles", bufs=1) as singles,
            tc.tile_pool(name="xrow", bufs=2) as xrow_pool,
            tc.tile_pool(name="apool", bufs=6) as apool,
            tc.tile_pool(name="psum", bufs=2, space="PSUM") as psum_pool,
        ):
            ones = singles.tile([1, P], F32)
            nc.vector.memset(ones[:], 1.0)

            # w = 1 - x replicated on all partitions. Piece 0 comes via a
            # small broadcast read from DRAM (2 MiB) so the DVE stream can
            # start ~9us in; pieces 1..3 are built off the critical path by
            # PE outer-product (fp32 matmul is 4x-slow, ~1.7us/512 cols) +
            # ACT copies fused with 1-x. x staging DMAs ride the ACT ring so
            # the sync ring carries nothing but the A stream.
            w_tiles = [
                singles.tile([P, XP], F32, name=f"w{i}", tag=f"w{i}")
                for i in range(N // XP)
            ]
            for piece in range(N // XP):
                xp = xrow_pool.tile([1, XP], F32, tag="xr")
                nc.scalar.dma_start(
                    out=xp[:], in_=x_row[:, piece * XP:(piece + 1) * XP]
                )
                wt = w_tiles[piece]
                for h in range(XP // PSB):
                    ps = psum_pool.tile([P, PSB], F32, tag="bc")
                    for j in range(PSB // BC):
                        col = h * PSB + j * BC
                        nc.tensor.matmul(
                            ps[:, j * BC:(j + 1) * BC],
                            ones[:],
                            xp[:, col:col + BC],
                            start=True,
                            stop=True,
                        )
                    nc.scalar.activation(
                        wt[:, h * PSB:(h + 1) * PSB],
                        ps[:],
                        mybir.ActivationFunctionType.Identity,
                        bias=1.0,
                        scale=-1.0,
                    )

            # x rows for this core: partition p gets x[p*NT:(p+1)*NT] (64 B).
            x_sb = singles.tile([P, NT], F32)
            nc.scalar.dma_start(
                out=x_sb[:], in_=x_s.rearrange("(p t) o -> p (t o)", t=NT)
            )

            NSLOT = NCH + 1
            acc = singles.tile([P, NT * NSLOT], F32)
            dummy = singles.tile([P, 1], F32)
            nc.vector.memset(acc[:], 0.0)

            def dot_chunk(t, c, lo, size, slot):
                at = apool.tile([P, size], F32, tag="A", name="at")
                nc.sync.dma_start(out=at[:], in_=A_r[t, :, lo:lo + size])
                # acc[:, slot] = sum_f (A * R) * w  (scale by R rides along)
                nc.vector.scalar_tensor_tensor(
                    out=dummy.broadcast_to([P, size]),
                    in0=at[:],
                    scalar=R_COEF,
                    in1=w_tiles[c][:, lo - c * CHUNK:lo - c * CHUNK + size],
                    op0=mybir.AluOpType.mult,
                    op1=mybir.AluOpType.mult,
                    accum_out=acc[:, slot:slot + 1],
                )

            # column-major: all row groups' chunk c before chunk c+1, so the
            # first 16 DVE ops need only w_tiles[0] (ready earliest). The
            # last two row groups' final chunks are halved so the DVE drains
            # quickly after the last DMA lands.
            for c in range(NCH):
                for t in range(NT):
                    if c == NCH - 1 and t >= NT - 2:
                        h = CHUNK // 2
                        dot_chunk(t, c, c * CHUNK, h, t * NSLOT + c)
                        dot_chunk(t, c, c * CHUNK + h, h, t * NSLOT + c + 1)
                    else:
                        dot_chunk(t, c, c * CHUNK, CHUNK, t * NSLOT + c)

            # reduce the partial sums per row: [P, NT, NSLOT] -> [P, NT]
            red = singles.tile([P, NT], F32)
            nc.vector.tensor_reduce(
                red[:],
                acc.rearrange("p (t c) -> p t c", c=NSLOT),
                axis=mybir.AxisListType.X,
                op=mybir.AluOpType.add,
            )

            # y = x * acc  (R already folded into the accumulation)
            y_sb = singles.tile([P, NT], F32)
            nc.vector.tensor_tensor(
                y_sb[:], x_sb[:], red[:], mybir.AluOpType.mult
            )
            nc.sync.dma_start(
                out=y_s.rearrange("(p t) o -> p (t o)", t=NT), in_=y_sb[:]
            )
    nc.compile()
    return nc


_NC = None


def _get_nc():
    global _NC
    if _NC is None:
        _NC = build()
    return _NC


def _in_maps(x, A):
    return [
        {
            "A_s": A[c * ROWS:(c + 1) * ROWS],
            "x_full": x,
            "x_s": x[c * ROWS:(c + 1) * ROWS],
        }
        for c in range(NCORES)
    ]


def run(t, x, A, **kw):
    """Run on the 8 NeuronCores; returns (y, BassKernelResults)."""
    x = np.ascontiguousarray(np.asarray(x, dtype=np.float32).reshape(N, 1))
    A = np.asarray(A, dtype=np.float32)
    res = run_bass_kernel_spmd(
        _get_nc(), _in_maps(x, A), list(range(NCORES)), **kw
    )
    y = np.concatenate(
        [np.asarray(res.results[c]["y_s"]) for c in range(NCORES)], axis=0
    )
    return y.astype(np.float32), res


def kernel(t, x, A):
    y, _ = run(t, x, A)
    return y
